# revision 1
# baseline (speedup 1.0000x reference)
"""Trainium2 Bass kernel for nn_CausalSelfAttention_38620345926298.

Sharding: 8 cores = 4 batches x 2 head-groups (8 heads each); partial output
projections of each core pair are summed on the host.

Device layout: attention is computed transposed -- attT[s, t] (key index s on
partitions, query index t on the free dim) -- so h^T, q^T, k^T and v are all
loaded/consumed in natural orientation and the kernel needs no on-device
transposes.

Per-core SPMD program:
  phase 1: q^T = (Wq/8)^T x^T, k^T = Wk^T x^T (c_out on partitions, q/k biases
           folded into the ACT PSUM->SBUF copy), v = x Wv (t on partitions)
           with two ones columns appended.
  phase 2: per head, pipelined per single-bank PSUM chunk: attT = k q^T (PE,
           fp32r), += h^T (DVE, bf16 h), *= blur masks (DVE, sub-regions),
           exp (ACT, PSUM->SBUF); y^T(66 rows) = [v|1|1]^T att_exp accumulated
           over s-tiles -- row 64 is the softmax denominator (ones column =>
           sum rides the same matmul). reciprocal (DVE) -> broadcast across 64
           partitions via a K=1 outer-product matmul -> y^T *= 1/sum.
  phase 3: out^T = Wp_slice^T y^T -> DRAM. Host: out = (pair sum)^T + bv@Wp+bp
           (exact: softmax rows sum to 1, so the v/proj biases are a constant
           output row).

float32r matmuls (full PE rate, ~2e-4 rel err) require even free-dim counts,
so the t/s axes are padded to TP=828; the padding row s=827 is killed via
h^T = -1e30 (exp -> 0) and the padding column t=827 is computed but never
stored. The causal mask is exact: the host sets the whole sub-diagonal of
h^T to -1e30. Softmax skips max-subtraction (logits are O(1); exp cannot
overflow). Known HW quirk: widening the fp32r qk matmul rhs below the
diagonal crashes for s>=5 (see WIDE_SET).
"""

import numpy as np

B, T, C = 4, 827, 1024
NH, HD = 16, 64
NCORES = 8
HPG = NH // 2          # heads per group (per core)
GW = HPG * HD          # group width = 512
PT = 128               # partition tile
TP = 828               # t/s axis padded even for fp32r matmuls
NT = (TP + PT - 1) // PT  # 7 t/s tiles
KT = C // PT           # 8 k tiles
BANK = 512             # psum bank, f32 elems
VW = HD + 2            # v row width incl. ones columns (66, even)
NEG = -1.0e30

F32R = True            # use float32r (full-rate) matmuls for the big GEMMs

_CACHE = {}


def _tsz(i):
    return min(PT, TP - i * PT)   # 128 x 6, 60


def _chunks(t0):
    """Bank-aligned free-dim chunks covering [t0, TP); all sizes even."""
    out = []
    if t0 < BANK:
        out.append((t0, BANK - t0))
        out.append((BANK, TP - BANK))
    else:
        out.append((t0, TP - t0))
    return out


WIDE = True
YNARROW = False
QNARROW = False
H_ON_DVE = True
H_BF16 = True
# Widening s>=4 (base 512 < t0) triggers a hardware fault in the fp32r qk
# matmul (bisected: lhsT offset 2560/3072 + rhs offset 2048 + dst 0 on the
# K=64 att matmul dies; same shapes at s=4 work). s=3 widening is verified.
WIDE_SET = frozenset([3])


def _base(t0):
    """Widened chunk start (>=256 sizes keep fp32r at full rate); columns in
    [base, t0) are sub-diagonal and get killed by h^T = -1e30 -> exp 0."""
    if not WIDE or (t0 // PT) not in WIDE_SET:
        return t0
    return min(t0, BANK - 256) if t0 < BANK else BANK


def _chunks_w(t0):
    b = _base(t0)
    if b < BANK:
        return [(b, BANK - b), (BANK, TP - BANK)]
    return [(b, TP - b)]


def _build_nc(loop_k=1):
    import concourse.tile as tile
    import concourse.mybir as mybir
    from concourse import bacc

    f32 = mybir.dt.float32
    mdt = mybir.dt.float32r if F32R else mybir.dt.float32

    nc = bacc.Bacc("TRN2", target_bir_lowering=False, debug=False,
                   num_devices=NCORES)

    xT = nc.dram_tensor("xT", [C, T], mdt, kind="ExternalInput").ap()
    wq = nc.dram_tensor("wq", [C, GW], mdt, kind="ExternalInput").ap()
    wk = nc.dram_tensor("wk", [C, GW], mdt, kind="ExternalInput").ap()
    wv = nc.dram_tensor("wv", [C, GW], mdt, kind="ExternalInput").ap()
    wp = nc.dram_tensor("wp", [GW, C], mdt, kind="ExternalInput").ap()
    bq = nc.dram_tensor("bq", [GW, 1], f32, kind="ExternalInput").ap()
    bk = nc.dram_tensor("bk", [GW, 1], f32, kind="ExternalInput").ap()
    hdt = mybir.dt.bfloat16 if H_BF16 else mdt
    hT = nc.dram_tensor("hT", [HPG, TP, TP], hdt, kind="ExternalInput").ap()
    m01 = nc.dram_tensor("m01", [2, PT, 256], f32, kind="ExternalInput").ap()
    m02 = nc.dram_tensor("m02", [2, PT, 256], f32, kind="ExternalInput").ap()
    m12 = nc.dram_tensor("m12", [3, PT, 256], f32, kind="ExternalInput").ap()
    # [:, 0:HD] = 1.0 (v ones cols, ones64 row), [:, HD] = 0.0 (x pad col)
    cst = nc.dram_tensor("cst", [PT, HD + 1], mdt,
                         kind="ExternalInput").ap()
    ident = nc.dram_tensor("ident", [PT, PT], mdt, kind="ExternalInput").ap()
    outT = nc.dram_tensor("outT", [C, T], f32, kind="ExternalOutput").ap()

    Exp = mybir.ActivationFunctionType.Exp

    def _emit(tc):
        with tc.tile_pool(name="persist", bufs=1) as persist:
            # ---- constants / persistent tiles ----
            ones64 = persist.tile([1, HD], mdt, tag="ones64")
            id_sb = persist.tile([PT, PT], mdt, tag="id_sb")
            wpt = [persist.tile([PT, C], mdt, name=f"wp{k}", tag=f"wp{k}")
                   for k in range(GW // PT)]
            msk = {}
            for mname, map_, nblk in (("m01", m01, 2), ("m02", m02, 2),
                                      ("m12", m12, 3)):
                for j in range(nblk):
                    mt = persist.tile([PT, 256], f32, name=f"{mname}_{j}",
                                      tag=f"{mname}_{j}")
                    msk[(mname, j)] = mt

            def persist_dmas():
                # emitted after the phase-1 input loads so they don't delay
                # the first projection matmuls
                nc.sync.dma_start(out=ones64[:], in_=cst[0:1, 0:HD])
                nc.sync.dma_start(out=id_sb[:], in_=ident[:])
                for mname, map_, nblk in (("m01", m01, 2), ("m02", m02, 2),
                                          ("m12", m12, 3)):
                    for j in range(nblk):
                        nc.sync.dma_start(out=msk[(mname, j)][:], in_=map_[j])
                for k in range(GW // PT):
                    nc.sync.dma_start(out=wpt[k][:],
                                      in_=wp[k * PT:(k + 1) * PT, :])

            qT = [persist.tile([PT, TP], mdt, name=f"qT{m}", tag=f"qT{m}")
                  for m in range(GW // PT)]
            kTt = [persist.tile([PT, TP], mdt, name=f"kT{m}", tag=f"kT{m}")
                   for m in range(GW // PT)]
            vt = [persist.tile([PT, HPG, VW], mdt, name=f"v{t}",
                               tag=f"v{t}") for t in range(NT)]
            yT = [persist.tile([PT, TP], mdt, name=f"yT{m}", tag=f"yT{m}")
                  for m in range(GW // PT)]

            # ================= phase 1: projections =================
            with tc.tile_pool(name="p1", bufs=1) as p1, \
                 tc.tile_pool(name="p1p", bufs=3, space="PSUM") as p1p, \
                 tc.tile_pool(name="p1vp", bufs=2, space="PSUM") as p1vp:
                xt = [p1.tile([PT, TP], mdt, name=f"xt{k}", tag=f"xt{k}")
                      for k in range(KT)]
                wts = {w: [p1.tile([PT, GW], mdt, name=f"{w}_{k}",
                                   tag=f"{w}_{k}") for k in range(KT)]
                       for w in ("wq", "wk", "wv")}
                for k in range(KT):
                    nc.sync.dma_start(out=xt[k][:, 0:T],
                                      in_=xT[k * PT:(k + 1) * PT, :])
                    nc.sync.dma_start(out=xt[k][:, T:TP],
                                      in_=cst[:, HD:HD + 1])
                    for wname, wap in (("wq", wq), ("wk", wk), ("wv", wv)):
                        nc.sync.dma_start(out=wts[wname][k][:],
                                          in_=wap[k * PT:(k + 1) * PT, :])
                bqs, bks = [], []
                for m in range(GW // PT):
                    bt = p1.tile([PT, 1], f32, name=f"bq_{m}", tag=f"bq_{m}")
                    nc.sync.dma_start(out=bt[:], in_=bq[m * PT:(m + 1) * PT, :])
                    bqs.append(bt)
                    bt2 = p1.tile([PT, 1], f32, name=f"bk_{m}", tag=f"bk_{m}")
                    nc.sync.dma_start(out=bt2[:], in_=bk[m * PT:(m + 1) * PT, :])
                    bks.append(bt2)
                for t in range(NT):
                    nc.sync.dma_start(
                        out=vt[t][:, :, HD:VW],
                        in_=cst[:, 0:2 * HPG].rearrange("p (h c) -> p h c",
                                                        h=HPG))
                persist_dmas()

                # q^T / k^T: out (128, TP) per m-tile, contraction over C
                for wname, dest, biases in (("wq", qT, bqs), ("wk", kTt, bks)):
                    for m in range(GW // PT):
                        ps = p1p.tile([PT, TP], f32, tag="proj")
                        for (c0, cn) in _chunks(0):
                            for k in range(KT):
                                nc.tensor.matmul(
                                    ps[:, c0:c0 + cn],
                                    wts[wname][k][:, m * PT:(m + 1) * PT],
                                    xt[k][:, c0:c0 + cn],
                                    start=(k == 0), stop=(k == KT - 1))
                        nc.scalar.add(dest[m][:], ps[:], biases[m][:])

                # v: out (tsz, 512) per t-tile
                for t in range(NT):
                    tsz = _tsz(t)
                    ps = p1vp.tile([PT, GW], f32, tag="vproj")
                    for k in range(KT):
                        nc.tensor.matmul(
                            ps[:tsz, :],
                            xt[k][:, t * PT:t * PT + tsz],
                            wts["wv"][k][:],
                            start=(k == 0), stop=(k == KT - 1))
                    nc.scalar.copy(
                        vt[t][:tsz, :, 0:HD],
                        ps[:tsz, :].rearrange("p (h d) -> p h d", h=HPG))

            # ================= phase 2: attention =================
            with tc.tile_pool(name="ht", bufs=10) as htp, \
                 tc.tile_pool(name="ax", bufs=4) as axp, \
                 tc.tile_pool(name="bc", bufs=2) as bcp, \
                 tc.tile_pool(name="rc", bufs=2) as rcp, \
                 tc.tile_pool(name="attp", bufs=4, space="PSUM") as attp, \
                 tc.tile_pool(name="yp", bufs=2, space="PSUM") as yp:

                def _mask_regions(s, c0, cn):
                    # (global_lo, global_hi, mask_tile, mask_col0, row_hi)
                    regs = []
                    if s in (0, 1):
                        regs.append((285, 541, msk[("m01", s)], 285, PT))
                        regs.append((571, T, msk[("m02", s)], 571, PT))
                    elif s in (2, 3):
                        regs.append((571, T, msk[("m12", s - 2)], 571, PT))
                    elif s == 4:
                        regs.append((571, T, msk[("m12", 2)], 571, 32))
                    out = []
                    for (lo, hi, mtile, m0, rhi) in regs:
                        a, b = max(lo, c0), min(hi, c0 + cn)
                        if a < b:
                            out.append((a, b, mtile, m0, rhi))
                    return out

                def att_stage(h, s, y_ps):
                    """One (head, s-tile), pipelined per single-bank chunk."""
                    mt, p0 = h // 2, (h % 2) * HD
                    ssz = _tsz(s)
                    t0 = s * PT
                    base = _base(t0)
                    ht_t = htp.tile([PT, TP], hdt, tag="ht")
                    nc.sync.dma_start(out=ht_t[:ssz, 0:TP - base],
                                      in_=hT[h, t0:t0 + ssz, base:TP])
                    a_sb = axp.tile([PT, TP], mdt, tag="ax")
                    for (c0, cn) in _chunks_w(t0):
                        a_ps = attp.tile([PT, BANK], f32, tag="att")
                        qc0 = max(c0, t0) if QNARROW else c0
                        nc.tensor.matmul(
                            a_ps[:ssz, qc0 - c0:cn],
                            kTt[mt][p0:p0 + HD, t0:t0 + ssz],
                            qT[mt][p0:p0 + HD, qc0:c0 + cn],
                            start=True, stop=(H_ON_DVE))
                        if H_ON_DVE:
                            nc.vector.tensor_add(
                                a_ps[:ssz, 0:cn], a_ps[:ssz, 0:cn],
                                ht_t[:ssz, c0 - base:c0 - base + cn])
                        else:
                            nc.tensor.matmul(
                                a_ps[:ssz, 0:cn],
                                id_sb[:ssz, :ssz],
                                ht_t[:ssz, c0 - base:c0 - base + cn],
                                start=False, stop=True)
                        for (a, b, mtile, m0, rhi) in _mask_regions(s, c0, cn):
                            nc.vector.tensor_mul(
                                a_ps[0:rhi, a - c0:b - c0],
                                a_ps[0:rhi, a - c0:b - c0],
                                mtile[0:rhi, a - m0:b - m0])
                        nc.scalar.activation(a_sb[:ssz, c0 - base:c0 - base + cn],
                                             a_ps[:ssz, 0:cn], Exp)
                        # bank 0 of y_ps last gets fed at s=3 (t0=384<512)
                        last_s = (BANK // PT - 1) if c0 < BANK else (NT - 1)
                        yc0 = max(c0, t0) if YNARROW else c0
                        ycn = cn - (yc0 - c0)
                        nc.tensor.matmul(
                            y_ps[:, yc0:yc0 + ycn],
                            vt[s][:ssz, h % HPG, :],
                            a_sb[:ssz, yc0 - base:yc0 - base + ycn],
                            start=(s == 0), stop=(s == last_s))

                def head_tail(h, y_ps):
                    mt, p0 = h // 2, (h % 2) * HD
                    recip = rcp.tile([1, TP], mdt, tag="rc")
                    with nc.allow_low_precision(reason="fp32r recip feeds "
                                                "full-rate fp32r bcast mm"):
                        nc.vector.reciprocal(recip[:], y_ps[HD:HD + 1, :])
                    b_sb = bcp.tile([HD, TP], f32, tag="bc")
                    for (c0, cn) in _chunks(0):
                        b_ps = attp.tile([HD, BANK], f32, tag="att")
                        nc.tensor.matmul(b_ps[:, 0:cn], ones64[:],
                                         recip[:, c0:c0 + cn],
                                         start=True, stop=True)
                        nc.scalar.copy(b_sb[:, c0:c0 + cn], b_ps[:, 0:cn])
                    nc.vector.tensor_mul(yT[mt][p0:p0 + HD, :], y_ps[0:HD, :],
                                         b_sb[:])

                for hp in range(HPG // 2):
                    hA, hB = 2 * hp, 2 * hp + 1
                    yA = yp.tile([VW, TP], f32, tag="y")
                    yB = yp.tile([VW, TP], f32, tag="y")
                    for s in range(NT):
                        att_stage(hA, s, yA)
                        att_stage(hB, s, yB)
                    head_tail(hA, yA)
                    head_tail(hB, yB)

            # ================= phase 3: output projection =================
            with tc.tile_pool(name="p3o", bufs=2) as p3o, \
                 tc.tile_pool(name="p3p", bufs=3, space="PSUM") as p3p:
                for m in range(C // PT):
                    ps = p3p.tile([PT, TP], f32, tag="op")
                    for (c0, cn) in _chunks(0):
                        for k in range(GW // PT):
                            nc.tensor.matmul(
                                ps[:, c0:c0 + cn],
                                wpt[k][:, m * PT:(m + 1) * PT],
                                yT[k][:, c0:c0 + cn],
                                start=(k == 0), stop=(k == GW // PT - 1))
                    ot = p3o.tile([PT, TP], f32, tag="ot")
                    nc.scalar.copy(ot[:], ps[:])
                    nc.sync.dma_start(out=outT[m * PT:(m + 1) * PT, :],
                                      in_=ot[:, 0:T])

    with tile.TileContext(nc) as tc:
        if loop_k > 1:
            with tc.For_i(0, loop_k, 1):
                _emit(tc)
        else:
            _emit(tc)

    nc.compile()
    return nc


# ---------------- host-side preprocessing ----------------

def _gauss_A():
    hx = np.arange(7, dtype=np.float32) - 3.0
    k1 = np.exp(-0.5 * (hx / 1.5) ** 2)
    k1 = (k1 / k1.sum()).astype(np.float32)
    A = np.zeros((16, 16), np.float32)
    for i in range(16):
        for u in range(7):
            p = i - 3 + u
            if p < 0:
                p = -p
            if p > 15:
                p = 30 - p
            A[i, p] += k1[u]
    return A


def _blurred_map(f, b_perm):
    # f, b_perm: (B, 256, 256) -> reference's _blurred_map in numpy
    A = _gauss_A()
    bi = (f * b_perm).reshape(B * 256, 16, 16)
    bl = np.einsum("ij,njk,lk->nil", A, bi, A, optimize=True).astype(np.float32)
    mn, mx = bl.min(), bl.max()
    bl = np.clip((bl - mn) / (mx - mn), 0.0, 1.0)
    return bl.reshape(B, 256, 256) * f * b_perm


def _h_cast(a):
    if H_BF16:
        import ml_dtypes
        return np.ascontiguousarray(a).astype(ml_dtypes.bfloat16)
    return np.ascontiguousarray(a)


def _prep_inputs(x, h, f01, f02, f12, b01, b02, b12,
                 Wq, bq, Wk, bk, Wv, bv, Wp, bp):
    blur01 = _blurred_map(f01, np.transpose(b01, (0, 2, 1)))
    blur02 = _blurred_map(f02, np.transpose(b02, (0, 2, 1)))
    blur12 = _blurred_map(f12, np.transpose(b12, (0, 2, 1)))

    # h^T padded to TP rows/cols; the whole sub-diagonal (t < s) plus the
    # padding row/column are -1e30 so exp kills everything non-causal,
    # including sub-diagonal columns the widened fp32r chunks compute.
    hTfull = np.full((B, NH, TP, TP), NEG, np.float32)
    hTfull[:, :, :T, :T] = np.transpose(h, (0, 1, 3, 2))
    tri = np.tril(np.ones((TP, TP), dtype=bool), -1)  # t < s
    hTfull[:, :, tri] = NEG
    # padding column t=827 stays finite (exp=1) so its softmax sum is nonzero
    # and the never-stored column produces no inf/NaN downstream
    hTfull[:, :, :, T] = 0.0

    cstv = np.zeros((PT, HD + 1), np.float32)
    cstv[:, 0:HD] = 1.0
    identv = np.eye(PT, dtype=np.float32)

    in_maps = []
    for c in range(NCORES):
        b, g = c // 2, c % 2
        sl = slice(g * GW, (g + 1) * GW)
        m12p = np.ones((384, 256), np.float32)
        m12p[30:286, :] = blur12[b].T
        in_maps.append({
            "xT": np.ascontiguousarray(x[b].T),
            "wq": np.ascontiguousarray(Wq[:, sl]) / 8.0,
            "wk": np.ascontiguousarray(Wk[:, sl]),
            "wv": np.ascontiguousarray(Wv[:, sl]),
            "wp": np.ascontiguousarray(Wp[sl, :]),
            "bq": (bq[sl] / 8.0).reshape(GW, 1).astype(np.float32),
            "bk": bk[sl].reshape(GW, 1).astype(np.float32),
            "hT": _h_cast(hTfull[b, g * HPG:(g + 1) * HPG]),
            "m01": np.ascontiguousarray(blur01[b].T.reshape(2, PT, 256)),
            "m02": np.ascontiguousarray(blur02[b].T.reshape(2, PT, 256)),
            "m12": np.ascontiguousarray(m12p.reshape(3, PT, 256)),
            "cst": cstv,
            "ident": identv,
        })
    return in_maps


def _postprocess(results, Wv_bias_row):
    out = np.empty((B, T, C), np.float32)
    for b in range(B):
        acc = results[2 * b]["outT"] + results[2 * b + 1]["outT"]
        out[b] = acc.T + Wv_bias_row
    return out


def kernel(**inputs):
    inputs = {k: np.asarray(v, dtype=np.float32) for k, v in inputs.items()}
    if "nc" not in _CACHE:
        _CACHE["nc"] = _build_nc()
    nc = _CACHE["nc"]

    in_maps = _prep_inputs(**inputs)
    from concourse import bass_utils
    res = bass_utils.run_bass_kernel_spmd(nc, in_maps,
                                          core_ids=list(range(NCORES)))
    row = inputs["bv"] @ inputs["Wp"] + inputs["bp"]
    return _postprocess(res.results, row.astype(np.float32))



# revision 21
# speedup vs baseline: 1.0923x; 1.0923x over previous
"""Trainium2 Bass kernel for nn_CausalSelfAttention_38620345926298.

Sharding: 8 cores = 4 batches x 2 head-groups (8 heads each); partial output
projections of each core pair are summed on the host.

Attention is computed transposed -- attT[s, t] (key index s on partitions,
query index t on the free dim) -- so E, q^T, k^T and v are all loaded in
natural orientation with no on-device transposes.

v2 design (all matmuls bf16, elementwise rebalanced across engines):
  The additive bias h never reaches the device: the host precomputes
  E = exp(h^T * region_masks) with the causal mask baked in as exact zeros
  (exp(-1e30) = 0), so softmax numerators are exp(qk * m) * E. On device:
    PE:   attT chunk = k q^T (bf16, f32 PSUM)
    DVE:  region multiply by blur masks (consolidated span tiles, 7/head)
    ACT:  exp PSUM -> SBUF bf16
    DVE/Pool: a2 = exp(qk*m) * E  (bf16 2x-mode on DVE; s=0,1 on Pool)
    PE:   y^T[66 rows] += [v|1|1]^T a2  (row 64 = softmax denominator)
  Tail per head: DVE reciprocal -> K=1 ones-matmul broadcast -> ACT copy ->
  DVE scale.  Host adds back bv@Wp + bp (softmax rows sum to 1).

Phase 1 runs the v-projection contraction-outer so PE starts on the first
x^T tile; q/k biases fold into the ACT PSUM->SBUF copy.  Phase 3 is the
output projection with PSUM->SBUF copies alternating ACT/DVE.
"""

import numpy as np

B, T, C = 4, 827, 1024
NH, HD = 16, 64
NCORES = 8
HPG = NH // 2          # heads per group (per core)
GW = HPG * HD          # group width = 512
PT = 128               # partition tile
TP = 828               # t/s axis padded even
NT = (TP + PT - 1) // PT  # 7 t/s tiles
KT = C // PT           # 8 k tiles
BANK = 512             # psum bank, f32 elems
VW = HD + 2            # v row width incl. ones columns (66, even)
NEG = -1.0e30

POOL_S = frozenset([0, 1])   # s-tiles whose E-multiply runs on the Pool engine

_CACHE = {}


def _tsz(i):
    return min(PT, TP - i * PT)   # 128 x 6, 60


def _chunks(t0):
    """Bank-aligned free-dim chunks covering [t0, TP)."""
    if t0 < BANK:
        return [(t0, BANK - t0), (BANK, TP - BANK)]
    return [(t0, TP - t0)]


def _build_nc(loop_k=1):
    import concourse.tile as tile
    import concourse.mybir as mybir
    from concourse import bacc

    f32 = mybir.dt.float32
    mdt = mybir.dt.bfloat16

    nc = bacc.Bacc("TRN2", target_bir_lowering=False, debug=False,
                   num_devices=NCORES)

    xT = nc.dram_tensor("xT", [C, T], mdt, kind="ExternalInput").ap()
    wq = nc.dram_tensor("wq", [C, GW], mdt, kind="ExternalInput").ap()
    wk = nc.dram_tensor("wk", [C, GW], mdt, kind="ExternalInput").ap()
    wv = nc.dram_tensor("wv", [C, GW], mdt, kind="ExternalInput").ap()
    wp = nc.dram_tensor("wp", [GW, C], mdt, kind="ExternalInput").ap()
    bq = nc.dram_tensor("bq", [GW, 1], f32, kind="ExternalInput").ap()
    bk = nc.dram_tensor("bk", [GW, 1], f32, kind="ExternalInput").ap()
    Et = nc.dram_tensor("Et", [HPG, TP, TP], mdt, kind="ExternalInput").ap()
    cm01 = nc.dram_tensor("cm01", [2, PT, 544], mdt,
                          kind="ExternalInput").ap()
    cm2 = nc.dram_tensor("cm2", [3, PT, 258], mdt, kind="ExternalInput").ap()
    # [:, 0:HD] = 1.0 (v ones cols, ones64 row), [:, HD] = 0.0 (x pad col)
    cst = nc.dram_tensor("cst", [PT, HD + 1], mdt, kind="ExternalInput").ap()
    outT = nc.dram_tensor("outT", [C, T], mdt, kind="ExternalOutput").ap()

    Exp = mybir.ActivationFunctionType.Exp

    def _emit(tc):
        with tc.tile_pool(name="persist", bufs=1) as persist:
            # ---- constants / persistent tiles ----
            wpt = [persist.tile([PT, C], mdt, name=f"wp{k}", tag=f"wp{k}")
                   for k in range(GW // PT)]
            msk = {}
            for j in range(2):
                mt_ = persist.tile([PT, 544], mdt, name=f"cm01_{j}",
                                   tag=f"cm01_{j}")
                msk[(0, j)] = mt_
            for j in range(3):
                mt_ = persist.tile([PT, 258], mdt, name=f"cm2_{j}",
                                   tag=f"cm2_{j}")
                msk[(2, j)] = mt_

            def persist_dmas():
                # emitted after the phase-1 input loads so they don't delay
                # the first projection matmuls
                for j in range(2):
                    nc.sync.dma_start(out=msk[(0, j)][:], in_=cm01[j])
                for j in range(3):
                    nc.sync.dma_start(out=msk[(2, j)][:], in_=cm2[j])

            qT = [persist.tile([PT, TP], mdt, name=f"qT{m}", tag=f"qT{m}")
                  for m in range(GW // PT)]
            kTt = [persist.tile([PT, TP], mdt, name=f"kT{m}", tag=f"kT{m}")
                   for m in range(GW // PT)]
            vt = [persist.tile([PT, HPG, VW], mdt, name=f"v{t}",
                               tag=f"v{t}") for t in range(NT)]
            yT = [persist.tile([PT, TP], mdt, name=f"yT{m}", tag=f"yT{m}")
                  for m in range(GW // PT)]

            # ========== phase 1 (v-projection) + fused phase 1/2 ==========
            xt = [persist.tile([PT, TP], mdt, name=f"xt{k}", tag=f"xt{k}")
                  for k in range(KT)]
            wts = {w: [persist.tile([PT, GW], mdt, name=f"{w}_{k}",
                                    tag=f"{w}_{k}") for k in range(KT)]
                   for w in ("wq", "wk", "wv")}
            # v weights + x first so the v matmuls start ASAP
            for k in range(KT):
                nc.sync.dma_start(out=wts["wv"][k][:],
                                  in_=wv[k * PT:(k + 1) * PT, :])
                nc.sync.dma_start(out=xt[k][:, 0:T],
                                  in_=xT[k * PT:(k + 1) * PT, :])
                nc.sync.dma_start(out=xt[k][:, T:TP],
                                  in_=cst[:, HD:HD + 1])
            for k in range(KT):
                for wname, wap in (("wq", wq), ("wk", wk)):
                    nc.sync.dma_start(out=wts[wname][k][:],
                                      in_=wap[k * PT:(k + 1) * PT, :])
            bqs, bks = [], []
            for m in range(GW // PT):
                bt = persist.tile([PT, 1], f32, name=f"bq_{m}", tag=f"bq_{m}")
                nc.sync.dma_start(out=bt[:], in_=bq[m * PT:(m + 1) * PT, :])
                bqs.append(bt)
                bt2 = persist.tile([PT, 1], f32, name=f"bk_{m}",
                                   tag=f"bk_{m}")
                nc.sync.dma_start(out=bt2[:], in_=bk[m * PT:(m + 1) * PT, :])
                bks.append(bt2)
            for t in range(NT):
                nc.sync.dma_start(
                    out=vt[t][:, :, HD:VW],
                    in_=cst[:, 0:2 * HPG].rearrange("p (h c) -> p h c",
                                                    h=HPG))
            persist_dmas()

            # v: contraction(k)-outer so PE starts on the first x tile;
            # PSUM->SBUF copies alternate ACT/DVE to halve the drain.
            with tc.tile_pool(name="p1vp", bufs=1, space="PSUM") as p1vp:
                vp = [p1vp.tile([PT, GW], f32, name=f"vp{t}",
                                tag=f"vp{min(t, 5)}") for t in range(NT)]

                def v_copy(t):
                    tsz = _tsz(t)
                    src_ap = vp[t][:tsz, :].rearrange("p (h d) -> p h d",
                                                      h=HPG)
                    if t % 2 == 0:
                        nc.scalar.copy(vt[t][:tsz, :, 0:HD], src_ap)
                    else:
                        nc.vector.tensor_copy(vt[t][:tsz, :, 0:HD], src_ap)

                # two waves (t=6 reuses the t=0 bank) so only 6 PSUM banks
                # are held and proj m0 can start while the copies drain
                for k in range(KT):
                    for t in range(6):
                        tsz = _tsz(t)
                        nc.tensor.matmul(
                            vp[t][:tsz, :],
                            xt[k][:, t * PT:t * PT + tsz],
                            wts["wv"][k][:],
                            start=(k == 0), stop=(k == KT - 1))
                v_copy(0)
                for k in range(KT):
                    tsz = _tsz(6)
                    nc.tensor.matmul(
                        vp[6][:tsz, :],
                        xt[k][:, 6 * PT:6 * PT + tsz],
                        wts["wv"][k][:],
                        start=(k == 0), stop=(k == KT - 1))
                for t in range(1, NT):
                    v_copy(t)

            # ===== fused q/k projection + attention =====
            # One PSUM ring (attp, 3 x 2-bank tiles) serves both the q/k
            # projection m-tiles and the attention a_ps tiles, so projection
            # matmuls interleave into the attention stream and fill the
            # PE gaps left by the elementwise chain.  y is a single 2-bank
            # accumulator drained to SBUF right after the last av.
            with tc.tile_pool(name="et", bufs=10) as etp, \
                 tc.tile_pool(name="ax", bufs=4) as axp, \
                 tc.tile_pool(name="ax2", bufs=6) as ax2p, \
                 tc.tile_pool(name="bc", bufs=2) as bcp, \
                 tc.tile_pool(name="rc", bufs=2) as rcp, \
                 tc.tile_pool(name="ys", bufs=2) as ysp, \
                 tc.tile_pool(name="attp", bufs=3, space="PSUM") as attp, \
                 tc.tile_pool(name="yp", bufs=1, space="PSUM") as yp:

                def proj(wname, dest, biases, m):
                    """q^T / k^T m-tile: contraction over C into one 2-bank
                    ring tile; the bias-add drain (ACT for q, DVE for k)
                    frees the slot."""
                    ps = attp.tile([PT, TP], f32, tag="att")
                    for (c0, cn) in _chunks(0):
                        for k in range(KT):
                            nc.tensor.matmul(
                                ps[:, c0:c0 + cn],
                                wts[wname][k][:, m * PT:(m + 1) * PT],
                                xt[k][:, c0:c0 + cn],
                                start=(k == 0), stop=(k == KT - 1))
                    nc.scalar.add(dest[m][:], ps[:], biases[m][:])

                def _mask_region(s, t0):
                    # (global_lo, global_hi, mask_tile, mask_col0)
                    if s in (0, 1):
                        return (284, 828, msk[(0, s)], 284)
                    if s in (2, 3, 4):
                        return (570, 828, msk[(2, s - 2)], 570)
                    return None

                def qk_stage(h, s):
                    """qk matmuls + one mask + one exp + one E-multiply for
                    (h, s) on a single 2-bank a_ps tile."""
                    mt, p0 = h // 2, (h % 2) * HD
                    ssz = _tsz(s)
                    t0 = s * PT
                    wd = TP - t0
                    et = etp.tile([PT, TP], mdt, tag="et")
                    nc.sync.dma_start(out=et[:ssz, 0:wd],
                                      in_=Et[h, t0:t0 + ssz, t0:TP])
                    a_ps = attp.tile([PT, TP], f32, tag="att")
                    for (c0, cn) in _chunks(t0):
                        nc.tensor.matmul(
                            a_ps[:ssz, c0 - t0:c0 - t0 + cn],
                            kTt[mt][p0:p0 + HD, t0:t0 + ssz],
                            qT[mt][p0:p0 + HD, c0:c0 + cn],
                            start=True, stop=True)
                    reg = _mask_region(s, t0)
                    if reg is not None:
                        (a, b, mtile, m0) = reg
                        a = max(a, t0)
                        nc.vector.tensor_mul(
                            a_ps[:ssz, a - t0:b - t0],
                            a_ps[:ssz, a - t0:b - t0],
                            mtile[:ssz, a - m0:b - m0])
                    a_sb = axp.tile([PT, TP], mdt, tag="ax")
                    nc.scalar.activation(a_sb[:ssz, 0:wd],
                                         a_ps[:ssz, 0:wd], Exp)
                    a2 = ax2p.tile([PT, TP], mdt, tag="ax2")
                    eng = nc.gpsimd if s in POOL_S else nc.vector
                    eng.tensor_mul(a2[:ssz, 0:wd], a_sb[:ssz, 0:wd],
                                   et[:ssz, 0:wd])
                    return a2

                def av_stage(h, s, y_ps, a2):
                    ssz = _tsz(s)
                    t0 = s * PT
                    for (c0, cn) in _chunks(t0):
                        # bank 0 of y_ps last gets fed at s=3 (t0=384<512)
                        last_s = (BANK // PT - 1) if c0 < BANK else (NT - 1)
                        nc.tensor.matmul(
                            y_ps[:, c0:c0 + cn],
                            vt[s][:ssz, h, :],
                            a2[:ssz, c0 - t0:c0 - t0 + cn],
                            start=(s == 0), stop=(s == last_s))

                def y_drain(h, y_ps):
                    # one split copy PSUM->SBUF right after the last av, so
                    # the y banks free ~1us later and the next head's av
                    # never stalls on them; normalization happens lazily in
                    # SBUF (see head_tail).
                    y_sb = ysp.tile([VW, TP], mdt, tag="ysb")
                    nc.scalar.copy(y_sb[:, 0:BANK], y_ps[:, 0:BANK])
                    nc.vector.tensor_copy(y_sb[:, BANK:TP], y_ps[:, BANK:TP])
                    return y_sb

                def head_tail(h, y_sb):
                    # DVE recip -> Pool partition-broadcast -> DVE bf16
                    # 2x-mode scale; all SBUF, fully off the matmul chain.
                    mt, p0 = h // 2, (h % 2) * HD
                    recip = rcp.tile([1, TP], mdt, tag="rc")
                    with nc.allow_low_precision(reason="bf16 softmax "
                                                "denominators"):
                        nc.vector.reciprocal(recip[:], y_sb[HD:HD + 1, :])
                    b_sb = bcp.tile([HD, TP], mdt, tag="bc")
                    nc.gpsimd.partition_broadcast(b_sb[:], recip[:])
                    nc.vector.tensor_mul(yT[mt][p0:p0 + HD, :],
                                         y_sb[0:HD, :], b_sb[:])

                # software-pipelined by two s-stages (av(s-2) after qk(s));
                # projection m-tile m+1 interleaves into heads 2m/2m+1;
                # each head's tail is deferred into the next head's stream.
                proj("wq", qT, bqs, 0)
                proj("wk", kTt, bks, 0)
                pending_tail = None
                for h in range(HPG):
                    if h == HPG - 2:
                        # output-projection weights, late enough not to
                        # delay the E-tile stream
                        for k in range(GW // PT):
                            nc.sync.dma_start(out=wpt[k][:],
                                              in_=wp[k * PT:(k + 1) * PT, :])
                    m_next = h // 2 + 1
                    y_ps = yp.tile([VW, TP], f32, tag="y")
                    hist = []
                    for s in range(NT):
                        hist.append(qk_stage(h, s))
                        if s == 0 and m_next < GW // PT:
                            proj("wq" if h % 2 == 0 else "wk",
                                 qT if h % 2 == 0 else kTt,
                                 bqs if h % 2 == 0 else bks, m_next)
                        if s == 1 and pending_tail is not None:
                            head_tail(*pending_tail)
                            pending_tail = None
                        if s >= 2:
                            av_stage(h, s - 2, y_ps, hist[s - 2])
                    for s in (NT - 2, NT - 1):
                        av_stage(h, s, y_ps, hist[s])
                    pending_tail = (h, y_drain(h, y_ps))
                head_tail(*pending_tail)

            # ================= phase 3: output projection =================
            with tc.tile_pool(name="p3o", bufs=2) as p3o, \
                 tc.tile_pool(name="p3p", bufs=3, space="PSUM") as p3p:
                for m in range(C // PT):
                    ps = p3p.tile([PT, TP], f32, tag="op")
                    for (c0, cn) in _chunks(0):
                        for k in range(GW // PT):
                            nc.tensor.matmul(
                                ps[:, c0:c0 + cn],
                                wpt[k][:, m * PT:(m + 1) * PT],
                                yT[k][:, c0:c0 + cn],
                                start=(k == 0), stop=(k == GW // PT - 1))
                    ot = p3o.tile([PT, TP], mdt, tag="ot")
                    if m % 2 == 0:
                        nc.scalar.copy(ot[:], ps[:])
                    else:
                        nc.vector.tensor_copy(ot[:], ps[:])
                    nc.sync.dma_start(out=outT[m * PT:(m + 1) * PT, :],
                                      in_=ot[:, 0:T])

    with tile.TileContext(nc) as tc:
        if loop_k > 1:
            with tc.For_i(0, loop_k, 1):
                _emit(tc)
        else:
            _emit(tc)

    nc.compile()
    return nc


# ---------------- host-side preprocessing ----------------

def _gauss_A():
    hx = np.arange(7, dtype=np.float32) - 3.0
    k1 = np.exp(-0.5 * (hx / 1.5) ** 2)
    k1 = (k1 / k1.sum()).astype(np.float32)
    A = np.zeros((16, 16), np.float32)
    for i in range(16):
        for u in range(7):
            p = i - 3 + u
            if p < 0:
                p = -p
            if p > 15:
                p = 30 - p
            A[i, p] += k1[u]
    return A


def _blurred_map(f, b_perm):
    # f, b_perm: (B, 256, 256) -> reference's _blurred_map in numpy
    A = _gauss_A()
    bi = (f * b_perm).reshape(B * 256, 16, 16)
    bl = np.einsum("ij,njk,lk->nil", A, bi, A, optimize=True).astype(np.float32)
    mn, mx = bl.min(), bl.max()
    bl = np.clip((bl - mn) / (mx - mn), 0.0, 1.0)
    return bl.reshape(B, 256, 256) * f * b_perm


def _bf16(a):
    import ml_dtypes
    return np.ascontiguousarray(a).astype(ml_dtypes.bfloat16)


def _prep_inputs(x, h, f01, f02, f12, b01, b02, b12,
                 Wq, bq, Wk, bk, Wv, bv, Wp, bp):
    blur01 = _blurred_map(f01, np.transpose(b01, (0, 2, 1)))
    blur02 = _blurred_map(f02, np.transpose(b02, (0, 2, 1)))
    blur12 = _blurred_map(f12, np.transpose(b12, (0, 2, 1)))

    # E = exp(h^T * region_masks) with the causal mask as exact zeros.
    # h^T[s, t]; regions (in transposed coords): m01 s 0:256 x t 285:541,
    # m02 s 0:256 x t 571:827, m12 s 286:542 x t 571:827.  Region multiply
    # happens BEFORE the causal -1e30 overwrite (matching the reference
    # order); the padding column t=827 stays exp(0)=1, the padding row
    # s=827 is causally zero.
    hTfull = np.full((B, NH, TP, TP), NEG, np.float32)
    hTfull[:, :, :T, :T] = np.transpose(h, (0, 1, 3, 2))
    for b in range(B):
        hTfull[b, :, 0:256, 285:541] *= blur01[b].T[None]
        hTfull[b, :, 0:256, 571:827] *= blur02[b].T[None]
        hTfull[b, :, 286:542, 571:827] *= blur12[b].T[None]
    tri = np.tril(np.ones((TP, TP), dtype=bool), -1)  # t < s
    hTfull[:, :, tri] = NEG
    hTfull[:, :, :, T] = 0.0
    Efull = np.exp(hTfull)

    # consolidated device mask tiles for the qk region multiplies:
    # cm01[j]: [PT, 544] covering t in [284, 828) for s-tile j (m01 block,
    # ones gap, m02 block); cm2[j]: [PT, 258] covering t in [570, 828) for
    # s-tiles 2,3,4 (m12 with 30-row padding offset).
    cm01v = np.ones((2, PT, 544), np.float32)
    cm2v = np.ones((3, PT, 258), np.float32)
    m12p = np.ones((384, 256), np.float32)

    cstv = np.zeros((PT, HD + 1), np.float32)
    cstv[:, 0:HD] = 1.0

    in_maps = []
    for c in range(NCORES):
        b, g = c // 2, c % 2
        sl = slice(g * GW, (g + 1) * GW)
        b01t, b02t, b12t = blur01[b].T, blur02[b].T, blur12[b].T
        cm01v[:] = 1.0
        cm01v[0, :, 1:257] = b01t[0:128]
        cm01v[1, :, 1:257] = b01t[128:256]
        cm01v[0, :, 287:543] = b02t[0:128]
        cm01v[1, :, 287:543] = b02t[128:256]
        m12p[:] = 1.0
        m12p[30:286, :] = b12t
        cm2v[:] = 1.0
        cm2v[0, :, 1:257] = m12p[0:128]
        cm2v[1, :, 1:257] = m12p[128:256]
        cm2v[2, :, 1:257] = m12p[256:384]
        in_maps.append({
            "xT": _bf16(x[b].T),
            "wq": _bf16(Wq[:, sl] / 8.0),
            "wk": _bf16(Wk[:, sl]),
            "wv": _bf16(Wv[:, sl]),
            "wp": _bf16(Wp[sl, :]),
            "bq": (bq[sl] / 8.0).reshape(GW, 1).astype(np.float32),
            "bk": bk[sl].reshape(GW, 1).astype(np.float32),
            "Et": _bf16(Efull[b, g * HPG:(g + 1) * HPG]),
            "cm01": _bf16(cm01v),
            "cm2": _bf16(cm2v),
            "cst": _bf16(cstv),
        })
    return in_maps


def _postprocess(results, Wv_bias_row):
    out = np.empty((B, T, C), np.float32)
    for b in range(B):
        acc = (results[2 * b]["outT"].astype(np.float32)
               + results[2 * b + 1]["outT"].astype(np.float32))
        out[b] = acc.T + Wv_bias_row
    return out


def kernel(**inputs):
    inputs = {k: np.asarray(v, dtype=np.float32) for k, v in inputs.items()}
    if "nc" not in _CACHE:
        _CACHE["nc"] = _build_nc()
    nc = _CACHE["nc"]

    in_maps = _prep_inputs(**inputs)
    from concourse import bass_utils
    res = bass_utils.run_bass_kernel_spmd(nc, in_maps,
                                          core_ids=list(range(NCORES)))
    row = inputs["bv"] @ inputs["Wp"] + inputs["bp"]
    return _postprocess(res.results, row.astype(np.float32))
